# revision 1
# baseline (speedup 1.0000x reference)
"""DiT block kernel for Trainium2, 8-core SPMD, no collectives.

Sharding: core c handles batch b = c//2, query-half qh = c%2 (2048 query
tokens). Host permutes each core's x so its query tokens are rows 0..2047;
K/V are computed on-core over all 4096 rows (attention is invariant to key
order). Output gathered on host. Host prep is layout/dtype only (bf16
weight casts, column packing).

The kernel is built around the ACT (scalar) engine being the bottleneck:
50M softmax exp elements/core = ~400us of ACT at 1 elem/lane/cycle, so
everything else is arranged to stream under a continuous exp pipeline.

Per-core math (E=384, NH=6, HD=64, FF=1536):
  AdaLN rows = cond @ [g1|be1|a1|g2|be2|a2] + bias (bias via K=1 matmul).
  LN scale/shift are folded into the QKV / FF1 weights:
    q = (xhat*s1 + t1) @ wq = xhat @ (s1 . wq) + (t1 @ wq)
  so LN1 emits plain transposed xhat; per-output-dim bias columns
  (cq/ck/cv, cf1) are added on the PSUM->SBUF copies. Weight folds run on
  the otherwise-idle GpSimd engine.
  rstd = exp(-0.5*ln(var+eps)) keeps every ACT call in ONE table set
  (natural_log_exp_and_others) - no ACT table reloads (exp and sqrt live
  in different sets; a reload costs ~2.7us on HW).
  Scores are exp'd with activation scale=1/8 (free affine); head pairs
  share the 128-deep contraction via tile_position row packing.
  attnT_unnorm[{d,sum},q] += V_aug^T @ exp(scoresT); row 64 = softmax
  denominators. Normalization: reciprocal on PSUM row 64, SBUF DMA hop to
  partition 0 (partition_broadcast is broken on HW for base != 0), GpSimd
  partition_broadcast, DVE multiply. PSUM rows are copied out early so
  the single PSUM accumulator frees before the next pair's first PV.
  Scheduling: LN1 tiles 8..31 / KT chunks / V tiles are fused into the
  first attention pair's kt loop so exp starts ~15us in; remaining QT
  chunks and the downstream block (wo+residual, LN2, FFN) of chunk qn-1
  are sliced into filler chunks emitted between kt iterations of chunk
  qn's 2nd/3rd pairs, keeping the PE/ACT streams dense (engine streams
  are FIFO - a blocked op stalls everything behind it on that engine).
  DMA queue order == emission order: adaln weights, x(q-half), qkv
  weights, x(kv-half) first; FF weights stream late (first used ~150us
  in, folded on the fly from small staged chunks).
"""

import os

os.environ.setdefault("MYCRO_LOCAL_CACHE", "1")

from contextlib import ExitStack

import numpy as np

import concourse.bacc as bacc
import concourse.mybir as mybir
from concourse.masks import make_identity
from concourse.tile import TileContext

F32 = mybir.dt.float32
BF16 = mybir.dt.bfloat16
AF = mybir.ActivationFunctionType
OP = mybir.AluOpType

E = 384
NH = 6
HD = 64
FF = 1536
EPS = 1e-5
NCH = E // 128
NFH = FF // 128
NPAIR = NH // 2


def build_kernel(S_kv=4096, S_q=2048):
    nc = bacc.Bacc("TRN2", target_bir_lowering=False)

    NKT = S_kv // 128
    NQT = S_q // 128
    QCH = 512 if S_q % 512 == 0 else S_q
    NQN = S_q // QCH
    KCH = 512 if S_kv % 512 == 0 else S_kv
    NV = S_kv // KCH
    KPC = KCH // 128          # kt tiles per KT chunk
    TQ = QCH // 128           # token tiles per q-chunk
    NG = (NKT - NQT + 7) // 8  # kv-half xp DMA chunks of 8 tiles
    HB = 512

    xp = nc.dram_tensor("xp", [S_kv, E], BF16, kind="ExternalInput")[:, :]
    cond_col = nc.dram_tensor("cond_col", [E, 1], BF16, kind="ExternalInput")[:, :]
    adaln_w = nc.dram_tensor("adaln_w", [E, 6 * E], BF16, kind="ExternalInput")[:, :]
    adab_row_d = nc.dram_tensor("adab_row", [1, 6 * E], BF16, kind="ExternalInput")[:, :]
    colpack_d = nc.dram_tensor("colpack", [128, 12 + NFH], F32, kind="ExternalInput")[:, :]
    wq_d = nc.dram_tensor("wq", [E, E], BF16, kind="ExternalInput")[:, :]
    wk_d = nc.dram_tensor("wk", [E, E], BF16, kind="ExternalInput")[:, :]
    wv_d = nc.dram_tensor("wv", [E, E], BF16, kind="ExternalInput")[:, :]
    wo_d = nc.dram_tensor("wo", [E, E], BF16, kind="ExternalInput")[:, :]
    ff1_d = nc.dram_tensor("ff1", [E, FF], BF16, kind="ExternalInput")[:, :]
    ff2_d = nc.dram_tensor("ff2", [FF, E], BF16, kind="ExternalInput")[:, :]
    ff2b_d = nc.dram_tensor("ff2b", [1, E], F32, kind="ExternalInput")[:, :]
    out_d = nc.dram_tensor("out", [S_q, E], F32, kind="ExternalOutput")[:, :]

    ctx = ExitStack()
    with TileContext(nc) as tc, ctx:
        root = ctx.enter_context(tc.tile_pool(name="root", bufs=1))

        ident = root.tile([128, 128], BF16)
        make_identity(nc, ident)
        ident1 = root.tile([1, 1], F32)
        nc.vector.memset(ident1, 1.0)
        ones_f = root.tile([1, 128], F32)
        nc.vector.memset(ones_f, 1.0)
        ones_bf = root.tile([1, 128], BF16)
        nc.vector.memset(ones_bf, 1.0)
        eps_t = root.tile([128, 1], F32)
        nc.vector.memset(eps_t, EPS)

        # ---- small packed params (persistent) ----
        colpack = root.tile([128, 12 + NFH], F32)
        nc.sync.dma_start(colpack, colpack_d)
        ln1w_c = colpack[:, 0:NCH]
        ln1b_c = colpack[:, NCH:2 * NCH]
        ln2w_c = colpack[:, 2 * NCH:3 * NCH]
        ln2b_c = colpack[:, 3 * NCH:4 * NCH]
        ff1b_c = colpack[:, 12:12 + NFH]
        cond_bf = root.tile([128, NCH, 1], BF16)
        nc.sync.dma_start(cond_bf, cond_col.rearrange("(c p) o -> p c o", p=128))

        # ---- persistent big tensors ----
        xq_keep = root.tile([128, NQT, E], BF16)
        KT = [[root.tile([128, KCH], BF16, tag=f"KT{c}_{n}", name=f"KT{c}_{n}")
               for n in range(NV)] for c in range(NCH)]
        QT = [[root.tile([128, QCH], BF16, tag=f"QT{c}_{n}", name=f"QT{c}_{n}")
               for n in range(NQN)] for c in range(NCH)]
        V_sb = [root.tile([128, NH, HD + 1], BF16, tag=f"V{m}", name=f"V{m}")
                for m in range(NKT)]
        for m in range(NKT):
            nc.vector.memset(V_sb[m][:, :, HD:HD + 1], 1.0)
        wo_bf = root.tile([64, NH, E], BF16)
        ff1s = root.tile([128, NCH, FF], BF16)
        ff2a = root.tile([128, NFH, E], BF16)
        ff1bias_c = root.tile([128, NFH], F32)
        scale1_c = root.tile([128, NCH], F32)
        shift1_c = root.tile([128, NCH], F32)
        scale2_c = root.tile([128, NCH], F32)
        shift2_c = root.tile([128, NCH], F32)
        cq_col = root.tile([128, NCH], F32)
        ck_col = root.tile([128, NCH], F32)
        cv_bc = root.tile([128, E], F32)
        fb_bf = root.tile([1, E], BF16)

        # long-lived working pools: fresh addresses, no overlap-deps with
        # the transient prologue staging below
        att_ctx = ExitStack()
        psb = att_ctx.enter_context(tc.tile_pool(name="ps_sb", bufs=3))
        nrm = att_ctx.enter_context(tc.tile_pool(name="nrm", bufs=1))
        attq_p = att_ctx.enter_context(tc.tile_pool(name="attq", bufs=2))
        pln = att_ctx.enter_context(tc.tile_pool(name="ln1", bufs=4))
        py1 = att_ctx.enter_context(tc.tile_pool(name="y1", bufs=1))
        pxb = att_ctx.enter_context(tc.tile_pool(name="xpb", bufs=2))
        sco = att_ctx.enter_context(tc.tile_pool(name="sco", bufs=2, space="PSUM"))
        acc = att_ctx.enter_context(tc.tile_pool(name="acc", bufs=1, space="PSUM"))

        y1T = [[py1.tile([128, KCH], BF16, tag=f"y1T{c}_{n}", name=f"y1T{c}_{n}")
                for n in range(NV)] for c in range(NCH)]

        # prologue PSUM pool (2 banks; closes before downstream pools open)
        pre_ctx = ExitStack()
        pre = pre_ctx.enter_context(tc.tile_pool(name="pre", bufs=2, space="PSUM"))
        wsc_ctx = ExitStack()
        wsc = wsc_ctx.enter_context(tc.tile_pool(name="wsc", bufs=1))
        wqs = wsc.tile([128, NCH, E], BF16)
        wks = wsc.tile([128, NCH, E], BF16)
        wvs = wsc.tile([128, NCH, E], BF16)

        # ---------- transient prologue staging ----------
        rows_ctx = ExitStack()
        rows = rows_ctx.enter_context(tc.tile_pool(name="rows", bufs=1))
        aro = rows.tile([1, E], F32)          # current AdaLN row
        adab_st = rows.tile([1, E], BF16)     # current AdaLN bias row
        row_stage = rows.tile([1, 512], F32)
        alpha1_b = rows.tile([64, E], F32)
        alpha2_b = rows.tile([128, E], F32)
        adaln_cols = rows.tile([128, 6, NCH], F32)
        g1p_c = rows.tile([128, NCH], F32)
        g2p_c = rows.tile([128, NCH], F32)
        ff2b_r = rows.tile([1, E], F32)
        shift1_bc = rows.tile([128, NCH], BF16)
        shift2_bc = rows.tile([128, NCH], BF16)

        wfold_ctx = ExitStack()
        wstg = wfold_ctx.enter_context(tc.tile_pool(name="wstg", bufs=1))
        wq_bf = wstg.tile([128, NCH, E], BF16)
        wk_bf = wstg.tile([128, NCH, E], BF16)
        wv_bf = wstg.tile([128, NCH, E], BF16)

        aw_ctx = ExitStack()
        awp = aw_ctx.enter_context(tc.tile_pool(name="awp", bufs=1))
        aw_r = adaln_w.rearrange("(c p) n -> p c n", p=128)

        # ---- SP DMA ring: adaln weights, x, qkv weights, x-kv, ff chunks
        nc.sync.dma_start(ff2b_r, ff2b_d)
        awjs = []
        for j in range(6):
            awj = awp.tile([128, NCH, E], BF16, tag="awj", name="awj")
            nc.sync.dma_start(awj, aw_r[:, :, j * E:(j + 1) * E])
            awjs.append(awj)
        half = NQT // 2
        nc.sync.dma_start(
            xq_keep[:, 0:half, :],
            xp[0:half * 128, :].rearrange("(g p) e -> p g e", p=128))
        nc.sync.dma_start(
            xq_keep[:, half:NQT, :],
            xp[half * 128:S_q, :].rearrange("(g p) e -> p g e", p=128))
        nc.sync.dma_start(wq_bf, wq_d.rearrange("(c p) n -> p c n", p=128))
        nc.sync.dma_start(wk_bf, wk_d.rearrange("(c p) n -> p c n", p=128))
        nc.sync.dma_start(wv_bf, wv_d.rearrange("(c p) n -> p c n", p=128))
        xpb = []
        NXG = (NKT - NQT + 3) // 4
        for g in range(NXG):
            r0 = S_q + g * 4 * 128
            nrow = min(4 * 128, S_kv - r0)
            xt = pxb.tile([128, 4, E], BF16, tag="xpb", name=f"xpb{g}")
            nc.sync.dma_start(
                xt[:, 0:nrow // 128, :],
                xp[r0:r0 + nrow, :].rearrange("(g p) e -> p g e", p=128))
            xpb.append(xt)

        # ---------- AdaLN chain: one row at a time ----------
        # row j: cond @ W_j + b_j (bias via K=1 matmul), -> columns
        for j in range(6):
            ps = pre.tile([1, E], F32, tag="pre1", name="adps")
            for k in range(NCH):
                nc.tensor.matmul(ps, cond_bf[:, k, :], awjs[j][:, k, :],
                                 start=(k == 0), stop=False)
            nc.sync.dma_start(adab_st, adab_row_d[:, j * E:(j + 1) * E])
            nc.tensor.matmul(ps, ones_bf[0:1, 0:1], adab_st,
                             start=False, stop=True)
            nc.scalar.copy(aro, ps)
            for c in range(NCH):
                pst = pre.tile([128, 128], F32, tag="pre1", name="adT")
                nc.tensor.transpose(pst[:, 0:1],
                                    aro[:, c * 128:(c + 1) * 128], ident1)
                nc.scalar.copy(adaln_cols[:, j, c:c + 1], pst[:, 0:1])
            if j == 2:
                psa = pre.tile([128, E], F32, tag="pre1", name="a1bc")
                nc.tensor.matmul(psa[0:64, :], ones_f[:, 0:64], aro,
                                 start=True, stop=True)
                nc.scalar.copy(alpha1_b, psa[0:64, :])
            if j == 5:
                psb2 = pre.tile([128, E], F32, tag="pre1", name="a2bc")
                nc.tensor.matmul(psb2, ones_f, aro, start=True, stop=True)
                nc.scalar.copy(alpha2_b, psb2)
                nc.gpsimd.tensor_tensor(fb_bf, ff2b_r, aro, OP.mult)
        aw_ctx.close()

        # column math + weight folds on GpSimd (DVE belongs to LN1)
        nc.gpsimd.tensor_scalar(g1p_c, adaln_cols[:, 0, :], 1.0, None, OP.add)
        nc.gpsimd.tensor_scalar(g2p_c, adaln_cols[:, 3, :], 1.0, None, OP.add)
        nc.gpsimd.tensor_tensor(scale1_c, g1p_c, ln1w_c, OP.mult)
        nc.gpsimd.tensor_tensor(scale2_c, g2p_c, ln2w_c, OP.mult)
        nc.gpsimd.tensor_tensor(shift1_c, g1p_c, ln1b_c, OP.mult)
        nc.gpsimd.tensor_tensor(shift1_c, shift1_c, adaln_cols[:, 1, :],
                                OP.add)
        nc.gpsimd.tensor_tensor(shift2_c, g2p_c, ln2b_c, OP.mult)
        nc.gpsimd.tensor_tensor(shift2_c, shift2_c, adaln_cols[:, 4, :],
                                OP.add)
        for c in range(NCH):
            nc.gpsimd.tensor_scalar(wqs[:, c, :], wq_bf[:, c, :],
                                    scale1_c[:, c:c + 1], None, OP.mult)
            nc.gpsimd.tensor_scalar(wks[:, c, :], wk_bf[:, c, :],
                                    scale1_c[:, c:c + 1], None, OP.mult)
            nc.gpsimd.tensor_scalar(wvs[:, c, :], wv_bf[:, c, :],
                                    scale1_c[:, c:c + 1], None, OP.mult)
        nc.gpsimd.tensor_copy(shift1_bc, shift1_c)
        nc.gpsimd.tensor_copy(shift2_bc, shift2_c)

        # bias rows from folded shifts: cq = shift1 @ wq etc.
        def shift_proj_row(shift_col, w_bf, row_out):
            ps = pre.tile([1, E], F32, tag="pre1", name="cproj")
            for c in range(NCH):
                nc.tensor.matmul(ps, shift_col[:, c:c + 1], w_bf[:, c, :],
                                 start=(c == 0), stop=(c == NCH - 1))
            nc.scalar.copy(row_out, ps)

        for w_bf, col in ((wq_bf, cq_col), (wk_bf, ck_col)):
            shift_proj_row(shift1_bc, w_bf, row_stage[:, 0:E])
            for c in range(NCH):
                pst = pre.tile([128, 128], F32, tag="pre1", name="cT")
                nc.tensor.transpose(pst[:, 0:1],
                                    row_stage[:, c * 128:(c + 1) * 128],
                                    ident1)
                nc.scalar.copy(col[:, c:c + 1], pst[:, 0:1])
        shift_proj_row(shift1_bc, wv_bf, row_stage[:, 0:E])
        psc = pre.tile([128, E], F32, tag="pre1", name="cvbc")
        nc.tensor.matmul(psc, ones_f, row_stage[:, 0:E], start=True, stop=True)
        nc.scalar.copy(cv_bc, psc)


        # ---------- LN1 / QKV building blocks ----------
        def ln1_tile(i, act_copies=False):
            if i < NQT:
                src = xq_keep[:, i, :]
            else:
                j = i - NQT
                src = xpb[j // 4][:, j % 4, :]
            st = pln.tile([128, 6], F32, tag="st", name="st")
            nc.vector.bn_stats(st, src)
            mv = pln.tile([128, 2], F32, tag="mv", name="mv")
            nc.vector.bn_aggr(mv, st)
            lnv = pln.tile([128, 1], F32, tag="lnv", name="lnv")
            nc.scalar.activation(lnv, mv[:, 1:2], AF.Ln, bias=eps_t)
            rstd = pln.tile([128, 1], F32, tag="rstd", name="rstd")
            nc.scalar.activation(rstd, lnv, AF.Exp, scale=-0.5)
            nmr = pln.tile([128, 1], F32, tag="nmr", name="nmr")
            nc.vector.tensor_scalar(nmr, mv[:, 0:1], rstd, -1.0,
                                    OP.mult, OP.mult)
            xh = pln.tile([128, E], BF16, tag="xh", name="xh")
            nc.vector.tensor_scalar(xh, src, rstd, nmr, OP.mult, OP.add)
            for c in range(NCH):
                pst = pre.tile([128, 128], BF16, tag="pre1", name="pst")
                nc.tensor.transpose(pst, xh[:, c * 128:(c + 1) * 128], ident)
                dst = y1T[c][i // KPC][:, (i % KPC) * 128:(i % KPC + 1) * 128]
                if act_copies and c >= 1:
                    nc.scalar.copy(dst, pst)
                else:
                    nc.vector.tensor_copy(dst, pst)

        def qt_chunk(c, n):
            ps = pre.tile([128, QCH], F32, tag="pre1", name="qtps")
            base = (n * QCH) // KCH
            off = (n * QCH) % KCH
            for k in range(NCH):
                nc.tensor.matmul(ps, wqs[:, k, c * 128:(c + 1) * 128],
                                 y1T[k][base][:, off:off + QCH],
                                 start=(k == 0), stop=(k == NCH - 1))
            nc.vector.tensor_scalar(QT[c][n], ps, cq_col[:, c:c + 1], None,
                                    OP.add)

        def kt_chunk(c, n):
            ps = pre.tile([128, KCH], F32, tag="pre1", name="ktps")
            for k in range(NCH):
                nc.tensor.matmul(ps, wks[:, k, c * 128:(c + 1) * 128],
                                 y1T[k][n], start=(k == 0), stop=(k == NCH - 1))
            nc.vector.tensor_scalar(KT[c][n], ps, ck_col[:, c:c + 1], None,
                                    OP.add)

        def v_tile(m):
            ps = pre.tile([128, E], F32, tag="pre1", name="vps")
            for k in range(NCH):
                nc.tensor.matmul(ps, y1T[k][m // KPC]
                                 [:, (m % KPC) * 128:(m % KPC + 1) * 128],
                                 wvs[:, k, :], start=(k == 0),
                                 stop=(k == NCH - 1))
            nc.vector.tensor_tensor(
                V_sb[m][:, :, 0:HD],
                ps.rearrange("p (h d) -> p h d", h=NH),
                cv_bc.rearrange("p (h d) -> p h d", h=NH), OP.add)

        # prebuild the first 8 LN1 tiles; the rest stream inside pair 0
        LOOK = min(8, NKT)
        for i in range(LOOK):
            ln1_tile(i)
        qt_chunk(0, 0)

        # ---------- attention ----------
        PVLAG = 2

        def attn_pair(qn, p, atq, fused_kv=False, fused_ln=False, filler=None):
            h0, h1 = 2 * p, 2 * p + 1
            pa = acc.tile([65, 2 * HB], F32, tag="pa", name="pa")
            exs = {}

            def emit_pv(k):
                ex = exs.pop(k)
                nc.tensor.matmul(pa[:, 0:QCH], V_sb[k][:, h0, :],
                                 ex[:, 0:QCH],
                                 start=(k == 0), stop=(k == NKT - 1))
                nc.tensor.matmul(pa[:, HB:HB + QCH], V_sb[k][:, h1, :],
                                 ex[:, HB:HB + QCH],
                                 start=(k == 0), stop=(k == NKT - 1))

            for kt in range(NKT):
                if fused_ln and kt + LOOK < NKT:
                    ln1_tile(kt + LOOK)
                if qn == 0 and kt % KPC == 0:
                    kt_chunk(p, kt // KPC)
                if fused_kv:
                    v_tile(kt)
                if filler and kt % 5 == 2 and filler:
                    filler.pop(0)()
                ks = slice((kt % KPC) * 128, (kt % KPC + 1) * 128)
                ktile = KT[p][kt // KPC]
                qtile = QT[p][qn]
                ss = sco.tile([128, 2 * HB], F32, tag="ss", name="ss")
                nc.tensor.matmul(ss[:, 0:QCH], ktile[0:64, ks],
                                 qtile[0:64, :],
                                 start=True, stop=True, tile_position=(0, 0))
                nc.tensor.matmul(ss[:, HB:HB + QCH], ktile[64:128, ks],
                                 qtile[64:128, :],
                                 start=True, stop=True, tile_position=(64, 0))
                ex = psb.tile([128, 2 * HB], BF16, tag="ex", name="ex")
                if QCH == HB:
                    nc.scalar.activation(ex, ss, AF.Exp, scale=0.125)
                else:
                    nc.scalar.activation(ex[:, 0:QCH], ss[:, 0:QCH], AF.Exp,
                                         scale=0.125)
                    nc.scalar.activation(ex[:, HB:HB + QCH],
                                         ss[:, HB:HB + QCH], AF.Exp,
                                         scale=0.125)
                exs[kt] = ex
                if kt >= PVLAG:
                    emit_pv(kt - PVLAG)
            for k in range(max(0, NKT - PVLAG), NKT):
                emit_pv(k)
            # free the PSUM accumulator quickly: recip row 64, copy rows
            # 0..63 to SBUF, then normalize off-PSUM
            srow = nrm.tile([65, 2 * HB], F32, tag="srow", name="srow")
            nc.vector.reciprocal(srow[64:65, 0:QCH], pa[64:65, 0:QCH])
            nc.vector.reciprocal(srow[64:65, HB:HB + QCH],
                                 pa[64:65, HB:HB + QCH])
            ta = nrm.tile([64, 2 * HB], F32, tag="ta", name="ta")
            nc.vector.tensor_copy(ta[:, 0:QCH], pa[0:64, 0:QCH])
            nc.vector.tensor_copy(ta[:, HB:HB + QCH], pa[0:64, HB:HB + QCH])
            if QCH == HB:
                nc.sync.dma_start(srow[0:1, :], srow[64:65, :])
            else:
                nc.sync.dma_start(srow[0:1, 0:QCH], srow[64:65, 0:QCH])
                nc.sync.dma_start(srow[0:1, HB:HB + QCH],
                                  srow[64:65, HB:HB + QCH])
            rbc = nrm.tile([64, 2 * HB], F32, tag="rbc", name="rbc")
            nc.gpsimd.partition_broadcast(rbc[:, 0:QCH], srow[0:1, 0:QCH])
            if QCH < HB:
                nc.gpsimd.partition_broadcast(rbc[:, HB:HB + QCH],
                                              srow[0:1, HB:HB + QCH])
            else:
                nc.gpsimd.partition_broadcast(rbc[:, HB:2 * HB],
                                              srow[0:1, HB:2 * HB])
            nc.vector.tensor_tensor(atq[:, h0, :], ta[:, 0:QCH],
                                    rbc[:, 0:QCH], OP.mult)
            nc.vector.tensor_tensor(atq[:, h1, :], ta[:, HB:HB + QCH],
                                    rbc[:, HB:HB + QCH], OP.mult)

        # ---------- downstream ----------
        state = {}

        def downstream_chunks(qn, atq):
            dsp = state["dsp"]
            dsb = state["dsb"]
            dwk = state["dwk"]
            r1q = dsb.tile([128, TQ, E], F32, tag="r1q", name="r1q")
            y2q = dsb.tile([128, NCH, QCH], BF16, tag="y2q", name="y2q")
            h1q = dsb.tile([128, NFH, QCH], BF16, tag="h1q", name="h1q")
            otq = dsb.tile([128, TQ, E], F32, tag="otq", name="otq")
            chunks = []

            def wo_ln2_t(t):
                ps = dsp.tile([128, E], F32, tag="dsp", name="wops")
                for h in range(NH):
                    nc.tensor.matmul(ps, atq[:, h, t * 128:(t + 1) * 128],
                                     wo_bf[:, h, :],
                                     start=(h == 0), stop=(h == NH - 1))
                nc.vector.tensor_tensor(r1q[:, t, :], ps,
                                        xq_keep[:, qn * TQ + t, :], OP.add)
                st = dwk.tile([128, 6], F32, tag="st2", name="st2")
                nc.vector.bn_stats(st, r1q[:, t, :])
                mv = dwk.tile([128, 2], F32, tag="mv2", name="mv2")
                nc.vector.bn_aggr(mv, st)
                lnv = dwk.tile([128, 1], F32, tag="lnv2", name="lnv2")
                nc.scalar.activation(lnv, mv[:, 1:2], AF.Ln, bias=eps_t)
                rstd = dwk.tile([128, 1], F32, tag="rstd2", name="rstd2")
                nc.scalar.activation(rstd, lnv, AF.Exp, scale=-0.5)
                nmr = dwk.tile([128, 1], F32, tag="nmr2", name="nmr2")
                nc.vector.tensor_scalar(nmr, mv[:, 0:1], rstd, -1.0,
                                        OP.mult, OP.mult)
                xh = dwk.tile([128, E], BF16, tag="xh2", name="xh2")
                nc.vector.tensor_scalar(xh, r1q[:, t, :], rstd, nmr,
                                        OP.mult, OP.add)
                for c in range(NCH):
                    pst = dsp.tile([128, 128], BF16, tag="dsp", name="pst2")
                    nc.tensor.transpose(pst, xh[:, c * 128:(c + 1) * 128],
                                        ident)
                    nc.vector.tensor_copy(
                        y2q[:, c, t * 128:(t + 1) * 128], pst)

            def ff1_m3(m0):
                for m in range(m0, m0 + 3):
                    ps = dsp.tile([128, QCH], F32, tag="dsp", name="f1ps")
                    for k in range(NCH):
                        nc.tensor.matmul(ps,
                                         ff1s[:, k, m * 128:(m + 1) * 128],
                                         y2q[:, k, :],
                                         start=(k == 0), stop=(k == NCH - 1))
                    nc.vector.tensor_scalar(h1q[:, m, :], ps,
                                            ff1bias_c[:, m:m + 1], 0.0,
                                            OP.add, OP.max)

            def ff2_t(t):
                ps = dsp.tile([128, E], F32, tag="dsp", name="f2ps")
                for k in range(NFH):
                    nc.tensor.matmul(ps, h1q[:, k, t * 128:(t + 1) * 128],
                                     ff2a[:, k, :], start=(k == 0), stop=False)
                nc.tensor.matmul(ps, ones_bf, fb_bf, start=False, stop=True)
                nc.vector.tensor_tensor(otq[:, t, :], ps, r1q[:, t, :], OP.add)
                if t == TQ - 1:
                    nc.sync.dma_start(
                        out_d[qn * QCH:(qn + 1) * QCH, :]
                        .rearrange("(t p) e -> p t e", p=128), otq)

            for t in range(TQ):
                chunks.append(lambda t=t: wo_ln2_t(t))
            for m0 in range(0, NFH, 3):
                chunks.append(lambda m0=m0: ff1_m3(m0))
            for t in range(TQ):
                chunks.append(lambda t=t: ff2_t(t))
            return chunks

        def open_downstream_pools():
            wsc_ctx.close()
            pre_ctx.close()
            state["dsb"] = att_ctx.enter_context(
                tc.tile_pool(name="dsb", bufs=1))
            state["dwk"] = att_ctx.enter_context(
                tc.tile_pool(name="dwk", bufs=2))
            state["dsp"] = att_ctx.enter_context(
                tc.tile_pool(name="dsp", bufs=2, space="PSUM"))

        # ---------- main loop ----------
        qt_fill = [lambda c=c, n=n: qt_chunk(c, n)
                   for n in range(1, NQN) for c in range(NCH)]
        atqs = [attq_p.tile([64, NH, QCH], BF16, tag="atq", name="atq")]
        attn_pair(0, 0, atqs[0], fused_kv=True, fused_ln=True)
        for p in range(1, NPAIR):
            qt_chunk(p, 0)
            attn_pair(0, p, atqs[0], filler=qt_fill)
        while qt_fill:
            qt_fill.pop(0)()
        # ---- streamed FF folds: ff1 (+cf1 bias), ff2*alpha2, wo*alpha1 ----
        ff1_r = ff1_d.rearrange("(c p) n -> p c n", p=128)
        ff2_r = ff2_d.rearrange("(c p) n -> p c n", p=128)
        wo_r = wo_d.rearrange("(h d) n -> d h n", h=NH)
        for fc in range(NFH // 4):
            ps = pre.tile([1, 512], F32, tag="pre1", name="cf1")
            for c in range(NCH):
                f1c = wstg.tile([128, 1, 512], BF16, tag="ff1st", name="ff1st")
                nc.sync.dma_start(
                    f1c, ff1_r[:, c:c + 1, fc * 512:(fc + 1) * 512])
                nc.tensor.matmul(ps, shift2_bc[:, c:c + 1], f1c[:, 0, :],
                                 start=(c == 0), stop=(c == NCH - 1))
                nc.gpsimd.tensor_scalar(
                    ff1s[:, c, fc * 512:(fc + 1) * 512], f1c[:, 0, :],
                    scale2_c[:, c:c + 1], None, OP.mult)
            nc.scalar.copy(row_stage, ps)
            for kk in range(4):
                k = fc * 4 + kk
                pst = pre.tile([128, 128], F32, tag="pre1", name="cf1T")
                nc.tensor.transpose(pst[:, 0:1],
                                    row_stage[:, kk * 128:(kk + 1) * 128],
                                    ident1)
                nc.scalar.copy(ff1bias_c[:, k:k + 1], pst[:, 0:1])
        nc.gpsimd.tensor_tensor(ff1bias_c, ff1bias_c, ff1b_c, OP.add)
        for k0 in range(0, NFH, 2):
            f2c = wstg.tile([128, 2, E], BF16, tag="ff2st", name="ff2st")
            nc.sync.dma_start(f2c, ff2_r[:, k0:k0 + 2, :])
            for k in (k0, k0 + 1):
                nc.gpsimd.tensor_tensor(ff2a[:, k, :], f2c[:, k - k0, :],
                                        alpha2_b, OP.mult)
        for h0_ in range(0, NH, 2):
            woc = wstg.tile([64, 2, E], BF16, tag="wofst", name="wofst")
            nc.sync.dma_start(woc, wo_r[:, h0_:h0_ + 2, :])
            for h in (h0_, h0_ + 1):
                nc.gpsimd.tensor_tensor(wo_bf[:, h, :], woc[:, h - h0_, :],
                                        alpha1_b, OP.mult)
        wfold_ctx.close()
        rows_ctx.close()

        for qn in range(1, NQN):
            atqs.append(attq_p.tile([64, NH, QCH], BF16, tag="atq",
                                    name="atq"))
            attn_pair(qn, 0, atqs[qn])
            if qn == 1:
                open_downstream_pools()
            chunks = downstream_chunks(qn - 1, atqs[qn - 1])
            for p in range(1, NPAIR):
                attn_pair(qn, p, atqs[qn], filler=chunks)
            while chunks:
                chunks.pop(0)()
        if NQN == 1:
            open_downstream_pools()
        for fn in downstream_chunks(NQN - 1, atqs[NQN - 1]):
            fn()
        att_ctx.close()

    nc.finalize()
    return nc


_NC_CACHE = {}


def _get_nc(S_kv, S_q):
    key = (S_kv, S_q)
    if key not in _NC_CACHE:
        _NC_CACHE[key] = build_kernel(S_kv, S_q)
    return _NC_CACHE[key]


def _bf16(a):
    import ml_dtypes
    return np.asarray(a, np.float32).astype(ml_dtypes.bfloat16)


def make_in_maps(inputs, n_cores=8, S=4096):
    """Shard FULL inputs into per-core input maps (host work: layout and
    dtype prep only)."""
    x = np.asarray(inputs["x"], np.float32)
    cond = np.asarray(inputs["cond"], np.float32)
    Sq = S // 2
    adaln_w = np.concatenate(
        [np.asarray(inputs[k], np.float32)
         for k in ("g1_w", "be1_w", "a1_w", "g2_w", "be2_w", "a2_w")], axis=1)
    adaln_b = np.concatenate(
        [np.asarray(inputs[k], np.float32)
         for k in ("g1_b", "be1_b", "a1_b", "g2_b", "be2_b", "a2_b")])[None, :]

    def cols(v, nch):
        return np.asarray(v, np.float32).reshape(nch, 128).T

    colpack = np.concatenate(
        [cols(inputs["ln1_w"], NCH), cols(inputs["ln1_b"], NCH),
         cols(inputs["ln2_w"], NCH), cols(inputs["ln2_b"], NCH),
         cols(inputs["ff1_b"], NFH)], axis=1)

    shared = {
        "adaln_w": np.ascontiguousarray(_bf16(adaln_w)),
        "adab_row": np.ascontiguousarray(_bf16(adaln_b)),
        "colpack": np.ascontiguousarray(colpack),
        "wq": _bf16(inputs["wq"]),
        "wk": _bf16(inputs["wk"]),
        "wv": _bf16(inputs["wv"]),
        "wo": _bf16(inputs["wo"]),
        "ff1": _bf16(inputs["ff1_w"]),
        "ff2": _bf16(inputs["ff2_w"]),
        "ff2b": np.asarray(inputs["ff2_b"], np.float32)[None, :],
    }
    in_maps = []
    for c in range(n_cores):
        b, qh = c // 2, c % 2
        xb = x[b]
        xpm = np.concatenate([xb[qh * Sq:(qh + 1) * Sq],
                              xb[(1 - qh) * Sq:(2 - qh) * Sq]], axis=0)
        m = dict(shared)
        m["xp"] = np.ascontiguousarray(_bf16(xpm))
        m["cond_col"] = np.ascontiguousarray(_bf16(cond[b].reshape(E, 1)))
        in_maps.append(m)
    return in_maps


def kernel(**inputs):
    from concourse.bass_utils import run_bass_kernel_spmd

    x = np.asarray(inputs["x"], np.float32)
    B, S, _ = x.shape
    Sq = S // 2
    nc = _get_nc(S, Sq)
    in_maps = make_in_maps(inputs, n_cores=8, S=S)
    res = run_bass_kernel_spmd(nc, in_maps, core_ids=list(range(8)))
    out = np.empty((B, S, E), np.float32)
    for c in range(8):
        b, qh = c // 2, c % 2
        out[b, qh * Sq:(qh + 1) * Sq] = res.results[c]["out"]
    return out

